# revision 34
# baseline (speedup 1.0000x reference)
# Trainium2 Bass kernel for nn_Encoder_36902359007512 (DA-RNN style encoder).
#
# Math identity used: the attention logits are pre + (h@Wh + c@Wc)[:, None],
# where the h/c contribution is constant across the softmax axis (D), so
# softmax(logits) == softmax(pre): the attention weights are INDEPENDENT of
# the recurrent state, and time-invariant. The model reduces to:
#   attn = softmax(pre),  pre[b,d] = sum_t x[b,t,d] * Wx[t]   (b_attn cancels)
#   input_weighted[b,t,:] = attn[b] * x[b,t,:]
#   plain LSTM over t with inputs input_weighted -> input_encoded = h_t
#
# Sigmoid is computed as (tanh(x/2)+1)/2 so the whole kernel uses only
# {exp, tanh} from the single ACT table set "exp_and_others" (no table
# reloads). The +1 and /2 are folded into scalar_tensor_tensor ops, the
# doubled state cs = 2c (tanh(c) = Tanh(0.5*cs) via ACT's free affine), the
# doubled hs = 2h (W_hh pre-scaled by 0.5 on the host; enc scaled by 0.5 on
# the host after gathering).
#
# Layouts (host pre/post-transposed; only HW time is what matters):
#   x_dev    [T, D+1, BS]   feature-major, row D=81 is ones (bias row for mm)
#   w_in out [T, D, BS]     -> host transposes to (B, T, D)
#   enc out  [T, H, BS]     (holds 2*h) -> host: *0.5 and transpose
# Biases b_ih+b_hh ride the ih-matmul as contraction row 81 against the ones
# row of x. Gate order is permuted to (f, i, o, g) on the host.

import os
import numpy as np

B_FULL, TM1, D, H = 32768, 9, 81, 128
NCORES = 8
BS = B_FULL // NCORES        # 4096 batch rows per core
G4 = 4 * H                   # 512
DP1 = D + 1                  # 82: D data rows + ones row

# gate permutation: pytorch (i,f,g,o) -> (o,i,g,f)
_PERM = np.concatenate([
    np.arange(384, 512),     # o
    np.arange(0, 128),       # i
    np.arange(256, 384),     # g
    np.arange(128, 256),     # f
])

# engine for the big w_in elementwise multiply: "vector" or "gpsimd"
W_IN_ENGINE = os.environ.get("W_IN_ENGINE", "gpsimd")
PRE_ENGINE = os.environ.get("PRE_ENGINE", "vector")
# "f32r" (1 cyc/row relaxed-precision PE mode) or "f32" (4 cyc/row exact)
MM_MODE = os.environ.get("MM_MODE", "f32r")
# repeat whole kernel on-device (timing only; >1 recomputes same outputs)
REPS = int(os.environ.get("KERNEL_REPS", "1"))
T2_ENGINE = os.environ.get("T2_ENGINE", "vector")

_BUILD_CACHE = {}


def build_nc(bs=BS, nb=512):
    """Build the per-core Bass program. All 8 cores run the same program
    (SPMD) on different batch shards."""
    key = (bs, nb, W_IN_ENGINE, PRE_ENGINE, MM_MODE, T2_ENGINE, REPS)
    if key in _BUILD_CACHE:
        return _BUILD_CACHE[key]

    import concourse.bass as bass  # noqa: F401
    import concourse.mybir as mybir
    import concourse.tile as tile
    from concourse import bacc

    fp32 = mybir.dt.float32
    AF = mybir.ActivationFunctionType
    OP = mybir.AluOpType
    ntiles = bs // nb

    nc = bacc.Bacc("TRN2", target_bir_lowering=False)

    mmdt = mybir.dt.float32r if MM_MODE == "f32r" else fp32
    x_d = nc.dram_tensor("x", (TM1, DP1, bs), mmdt, kind="ExternalInput")
    wih_d = nc.dram_tensor("wih", (DP1, G4), mmdt, kind="ExternalInput")
    whh_d = nc.dram_tensor("whh", (H, G4), mmdt, kind="ExternalInput")
    wx_d = nc.dram_tensor("wx", (D, TM1), fp32, kind="ExternalInput")
    ones_d = nc.dram_tensor("onesw", (D, D), fp32, kind="ExternalInput")
    wiw_d = nc.dram_tensor("w_in", (TM1, D, bs), mmdt, kind="ExternalOutput")
    enc_d = nc.dram_tensor("enc", (TM1, H, bs), mmdt, kind="ExternalOutput")

    with tile.TileContext(nc) as tc:
        with (
            tc.tile_pool(name="const", bufs=1) as constp,
            tc.tile_pool(name="xw", bufs=4) as xwp,
            tc.tile_pool(name="attnw", bufs=3) as attnp,
            tc.tile_pool(name="cellw", bufs=6) as cellp,
            tc.tile_pool(name="cstate", bufs=ntiles) as csp,
            tc.tile_pool(name="hstate", bufs=8) as hp,
            tc.tile_pool(name="ps", bufs=2, space="PSUM") as psp,
        ):
            wih = constp.tile([DP1, G4], mmdt)
            nc.sync.dma_start(wih[:], wih_d[:])
            whh = constp.tile([H, G4], mmdt)
            nc.sync.dma_start(whh[:], whh_d[:])
            wx = constp.tile([D, TM1], fp32)
            nc.sync.dma_start(wx[:], wx_d[:])
            onesw = constp.tile([D, D], fp32)
            nc.sync.dma_start(onesw[:], ones_d[:])

            w_in_eng = getattr(nc, W_IN_ENGINE)
            pre_eng = getattr(nc, PRE_ENGINE)
            t2_eng = getattr(nc, T2_ENGINE)

            def attention(j):
                js, je = j * nb, (j + 1) * nb
                xw_j = xwp.tile([DP1, TM1, nb], mmdt, name=f"xw{j}", tag="xw")
                nc.sync.dma_start(
                    xw_j[:], x_d[:, :, js:je].rearrange("t d b -> d t b")
                )

                # pre[d,b] = sum_t Wx[t] * x[t,d,b]
                pre = attnp.tile([D, nb], fp32, name="pre", tag="pre")
                pre_eng.tensor_scalar_mul(pre[:], xw_j[:D, 0, :], wx[:, 0:1])
                for t in range(1, TM1):
                    pre_eng.scalar_tensor_tensor(
                        pre[:], xw_j[:D, t, :], wx[:, t : t + 1], pre[:],
                        op0=OP.mult, op1=OP.add,
                    )

                # eP = exp(pre); S = colsum(eP) broadcast to D partitions
                eP = attnp.tile([D, nb], fp32, name="eP", tag="eP")
                nc.scalar.activation(eP[:], pre[:], AF.Exp)
                S = psp.tile([128, 2048], fp32, name="S", tag="ps")
                nc.tensor.matmul(S[:D, :nb], onesw[:], eP[:], start=True, stop=True)
                # r = 1/S in place; attn = eP * r  (into pre's storage)
                nc.vector.reciprocal(S[:D, :nb], S[:D, :nb])
                nc.vector.tensor_mul(pre[:], eP[:], S[:D, :nb])

                # w_in[d,t,b] = attn[d,b] * x[d,t,b]  (in place in xw_j)
                w_in_eng.tensor_tensor(
                    xw_j[:D], xw_j[:D],
                    pre[:, None, :].to_broadcast((D, TM1, nb)),
                    op=OP.mult,
                )
                nc.sync.dma_start(
                    wiw_d[:, :, js:je].rearrange("t d b -> d t b"), xw_j[:D]
                )
                return xw_j

            def lstm_step(j, t, xw_j, cs_j, h_prev):
                js, je = j * nb, (j + 1) * nb
                gates = psp.tile([128, 2048], fp32, name="gates", tag="ps")
                # ih matmuls don't depend on h -> emit first (off the
                # recurrence cycle); hh matmuls for (f,i,g) before o since
                # tanh(o) is consumed late.
                for gc in range(4):
                    if t == 0 and gc == 3:
                        continue  # forget gate unused at t=0 (c0 == 0)
                    out_sl = gates[:, gc * 512 : (gc + 1) * 512][:, :nb]
                    nc.tensor.matmul(
                        out_sl, wih[:, gc * 128 : (gc + 1) * 128],
                        xw_j[:, t, :], start=True, stop=(t == 0),
                    )
                if t > 0:
                    for gc in (3, 1, 2, 0):  # f first, then i,g (t2 path), o last
                        out_sl = gates[:, gc * 512 : (gc + 1) * 512][:, :nb]
                        nc.tensor.matmul(
                            out_sl, whh[:, gc * 128 : (gc + 1) * 128],
                            h_prev[:], start=False, stop=True,
                        )

                # tanh(f/2) split out so stt1 (on the recurrence cycle) can
                # start as soon as the f-bank hh matmul lands; tanh((i,g')/2)
                # next (g pre-doubled on host so it yields tanh(g));
                # tanh(o/2) last — only needed late in the chain.
                if t > 0:
                    tf_t = cellp.tile([H, nb], fp32, name="tf", tag="tf")
                    nc.scalar.activation(
                        tf_t[:], gates[:, 3 * 512 :][:, :nb], AF.Tanh, scale=0.5,
                    )
                tig = cellp.tile([H, 2, nb], fp32, name="tig", tag="tig")
                nc.scalar.activation(
                    tig[:],
                    gates[:, 512 : 3 * 512].rearrange("p (g b) -> p g b", g=2)[
                        :, :, :nb
                    ],
                    AF.Tanh, scale=0.5,
                )
                to_t = cellp.tile([H, nb], fp32, name="to", tag="to")
                nc.scalar.activation(
                    to_t[:], gates[:, 0:512][:, :nb], AF.Tanh, scale=0.5,
                )

                if t == 0:
                    # cs = (tanh(i/2)+1) * tg  == 2*(sig(i)*tanh(g))
                    t2_eng.scalar_tensor_tensor(
                        cs_j[:], tig[:, 0, :], 1.0, tig[:, 1, :],
                        op0=OP.add, op1=OP.mult,
                    )
                else:
                    t1 = cellp.tile([H, nb], fp32, name="t1", tag="t1")
                    nc.vector.scalar_tensor_tensor(
                        t1[:], tf_t[:], 1.0, cs_j[:],
                        op0=OP.add, op1=OP.mult,
                    )
                    t2 = cellp.tile([H, nb], fp32, name="t2", tag="t2")
                    t2_eng.scalar_tensor_tensor(
                        t2[:], tig[:, 0, :], 1.0, tig[:, 1, :],
                        op0=OP.add, op1=OP.mult,
                    )
                    nc.vector.scalar_tensor_tensor(
                        cs_j[:], t1[:], 0.5, t2[:], op0=OP.mult, op1=OP.add,
                    )

                # tc = tanh(c) = Tanh(0.5 * cs)
                tc_t = cellp.tile([H, nb], fp32, name="tc", tag="tc")
                nc.scalar.activation(tc_t[:], cs_j[:], AF.Tanh, scale=0.5)
                # hs = (tanh(o/2)+1)*tc == 2*h ; whh is pre-scaled by 0.5
                h_t = hp.tile([H, nb], mmdt, name="h", tag="h")
                nc.vector.scalar_tensor_tensor(
                    h_t[:], to_t[:], 1.0, tc_t[:],
                    op0=OP.add, op1=OP.mult,
                )
                nc.sync.dma_start(enc_d[t, :, js:je], h_t[:])
                return h_t

            # Pairs of batch tiles interleaved step-by-step so the two PSUM
            # slots ping-pong between two independent recurrence chains.
            import contextlib
            rep_ctx = tc.For_i(0, REPS, 1) if REPS > 1 else contextlib.nullcontext()
            with rep_ctx:
              pairs = [
                  [j for j in (pj, pj + 1) if j < ntiles]
                  for pj in range(0, ntiles, 2)
              ]
              xws, css = {}, {}

              def emit_attention(pair):
                  for j in pair:
                      xws[j] = attention(j)
                      css[j] = csp.tile([H, nb], fp32, name=f"cs{j}", tag="cs")

              emit_attention(pairs[0])
              for pi, pair in enumerate(pairs):
                  # prefetch next pair's attention before this pair's LSTM so
                  # it overlaps and the pair transition has no bubble
                  if pi + 1 < len(pairs):
                      emit_attention(pairs[pi + 1])
                  hs = {j: None for j in pair}
                  for t in range(TM1):
                      for j in pair:
                          hs[j] = lstm_step(j, t, xws[j], css[j], hs[j])

    nc.compile()
    _BUILD_CACHE[key] = nc
    return nc


def prep_weights(W_attn, W_ih, W_hh, b_ih, b_hh):
    W_attn = np.asarray(W_attn, np.float32)
    W_ih = np.asarray(W_ih, np.float32)
    W_hh = np.asarray(W_hh, np.float32)
    b = np.asarray(b_ih, np.float32) + np.asarray(b_hh, np.float32)

    Wih_p = W_ih[_PERM]                     # (512, 81)
    Whh_p = W_hh[_PERM]                     # (512, 128)
    b_p = b[_PERM]                          # (512,)

    wih = np.zeros((DP1, G4), np.float32)
    wih[:D] = Wih_p.T
    wih[D] = b_p
    whh = np.ascontiguousarray(0.5 * Whh_p.T)          # hs = 2h folds 0.5 here
    # double the g-gate columns so Tanh(0.5*gates) == tanh(g) there
    wih[:, 256:384] *= 2.0
    whh[:, 256:384] *= 2.0
    wx = np.tile(W_attn[2 * H :].astype(np.float32)[None, :], (D, 1))  # (81, 9)
    onesw = np.ones((D, D), np.float32)
    return wih, whh, wx, onesw


def prep_x_shard(x_shard):
    # (bs, T, D) -> (T, D+1, bs) with ones row
    bs = x_shard.shape[0]
    xT = np.ascontiguousarray(np.transpose(x_shard, (1, 2, 0)), dtype=np.float32)
    out = np.empty((TM1, DP1, bs), np.float32)
    out[:, :D] = xT
    out[:, D] = 1.0
    return out


_LAST_RESULTS = None  # BassKernelResults of last device run (for test harness)


def kernel(input_data, W_attn, b_attn, W_ih, W_hh, b_ih, b_hh):
    global _LAST_RESULTS
    from concourse.bass_utils import run_bass_kernel_spmd

    x = np.asarray(input_data, np.float32)
    assert x.shape == (B_FULL, TM1, D), x.shape
    wih, whh, wx, onesw = prep_weights(W_attn, W_ih, W_hh, b_ih, b_hh)

    nc = build_nc()
    in_maps = []
    for c in range(NCORES):
        xs = prep_x_shard(x[c * BS : (c + 1) * BS])
        in_maps.append({"x": xs, "wih": wih, "whh": whh, "wx": wx, "onesw": onesw})

    trace = bool(int(os.environ.get("KERNEL_TRACE", "0")))
    try:
        res = run_bass_kernel_spmd(
            nc, in_maps, core_ids=list(range(NCORES)), trace=trace,
        )
    except ModuleNotFoundError:
        # NTFF profiling hook unavailable in this deployment
        res = run_bass_kernel_spmd(
            nc, in_maps, core_ids=list(range(NCORES)), trace=False,
        )
    _LAST_RESULTS = res

    iw = np.concatenate(
        [res.results[c]["w_in"].transpose(2, 0, 1) for c in range(NCORES)], axis=0
    )
    enc = np.concatenate(
        [res.results[c]["enc"].transpose(2, 0, 1) for c in range(NCORES)], axis=0
    )
    enc *= 0.5  # device stores hs = 2h
    return iw, enc

